# revision 19
# baseline (speedup 1.0000x reference)
"""Trainium2 Bass kernel for ChannelFeatures (channel-attention style module).

Computes, per batch element b:
    x_max[b] = max over (H,W) of features[b]          # (C,)
    x_avg[b] = mean over (H,W) of features[b]         # (C,)
    7 residual blocks (shared weights on both branches):
        x = prelu(W1[k] @ x + b1[k], a1[k]) + x
    scores[b] = sigmoid(x_max[b] + x_avg[b])          # (C,)
    out[b] = features[b] * scores[b]                  # broadcast over (H,W)

Sharding: pure data parallel over batch — 16 batch elements across 8 cores,
2 per core, weights replicated. No cross-core communication.

Device strategy per core (2 batch elements, each (65536, 64) fp32):
  HBM traffic is the 2x33.5MB floor (each element loaded once, stored once).
  Loads stream fp32 tiles on the sync HWDGE ring into a small rotating pool;
  ACT casts each tile to bf16 — that cast is BOTH the mean-matmul operand
  (PE ones-matmul accumulation, 1/HW folded into the final K=1 matmul) AND
  the resident pass-2 copy, so both batches fit in SBUF double-buffered and
  a load never waits on a store.
  Max: contiguous DVE bf16 max-folds over the cached casts (2x DVE rate;
  rounding error verified well under the tolerance), in-place binary tree,
  PE transpose + DVE reduce for the cross-partition max. The next batch's
  fold chain is gated on this batch's score row via a 1-element token write
  into the shared accumulator, so the serial recurrence is never dilated by
  interleaved full-tile folds.
  Recurrence: channels on partitions, (C, 2) tile; 7x (PE matmul + fused
  DVE prelu: z = y+b; (z max 0)+x; (a*z min 0); sum).
  Scores broadcast stays on-chip: PE transpose to a row, K=1 ones outer
  product to a (P, C) PSUM block, DVE materializes the (P, KF, C) bf16
  multiplier (no DRAM bounce, no ACT queueing behind the next batch's
  casts).
  Pass 2: bf16 multiply in place (2x DVE rate), store via the gpsimd SWDGE
  ring with a bf16->fp32 cast in the DMA datapath. Loads (sync ring) and
  stores (gpsimd ring) interleave at packet granularity, keeping HBM busy
  through each batch's recurrence gap.
"""

import numpy as np
from contextlib import ExitStack

import concourse.bass as bass
import concourse.tile as tile
from concourse import masks, mybir
from concourse.bass_utils import run_bass_kernel_spmd

# Problem shapes (hardcoded per contract)
B, H, W, C = 16, 256, 256, 64
CONV_NUM = 7
NCORES = 8
BPC = B // NCORES          # batch elements per core
HW = H * W                 # 65536 spatial positions
P = 128                    # SBUF partitions
KF = 64                    # spatial rows per partition per tile
TILE_ROWS = P * KF         # 4096 spatial rows per tile
T = HW // TILE_ROWS        # 16 tiles per batch element
F32 = mybir.dt.float32

# test.py hooks: set PROFILE=True before calling kernel() to capture an NTFF
# trace; LAST_EXEC_NS then holds the max per-core HW execution time.
PROFILE = False
LAST_EXEC_NS = None
LAST_RESULTS = None


def _split_dma_waits(nc: bass.Bass) -> None:
    """The pinned walrus build rejects DMA instructions carrying more than one
    sync-wait ("Too many sync wait commands"). Tile's sem assignment is not
    transitively minimal, so slot-reuse instructions can get two waits
    (consumer release + WAW with the previous writer). Hoist all but the last
    wait onto wait-only EventSemaphore instructions on the same engine right
    before the instruction."""
    n = 0
    # num=200: outside every id Tile allocated (its end-of-kernel range-clear
    # covers the allocated block), so no collision with released Tile sems.
    dummy = nc.alloc_semaphore(name="wsplit_dummy", num=200)
    for fn in nc.m.functions:
        for blk in fn.blocks:
            new_insts = []
            for inst in blk.instructions:
                si = getattr(inst, "sync_info", None)
                if si is not None and len(si.on_wait) > 1:
                    for w in si.on_wait[:-1]:
                        ev = mybir.InstEventSemaphore(
                            name=f"WSPLIT-{n}", ins=[], outs=[]
                        )
                        n += 1
                        ev.engine = inst.engine
                        # Tick a dedicated dummy sem nobody waits on, so the
                        # simulator/race tooling (which require every
                        # instruction to carry an update) accept the carrier.
                        upd = mybir.SyncUpdate(
                            sync_type="semaphore",
                            id=dummy.num,
                            ant_name=dummy.name,
                            update_mode="sem-add-imm",
                            update_value=1,
                        )
                        ev.sync_info = mybir.SyncInfo(on_wait=[w], on_update=[upd])
                        new_insts.append(ev)
                    si.on_wait = [si.on_wait[-1]]
                new_insts.append(inst)
            blk.instructions = new_insts


def _build_nc() -> bass.Bass:
    nc = bass.Bass()
    feat = nc.declare_dram_parameter("features", [BPC, HW, C], F32, isOutput=False)
    wT = nc.declare_dram_parameter("wT", [C, CONV_NUM, C], F32, isOutput=False)
    bT = nc.declare_dram_parameter("bT", [C, CONV_NUM], F32, isOutput=False)
    aT = nc.declare_dram_parameter("aT", [C, CONV_NUM], F32, isOutput=False)
    out = nc.declare_dram_parameter("out", [BPC, HW, C], F32, isOutput=True)

    feat_t = feat[:].rearrange("b (t p k) c -> b t p k c", p=P, k=KF)
    out_t = out[:].rearrange("b (t p k) c -> b t p k c", p=P, k=KF)

    SEG = KF // 8            # 512-wide matmul segments per tile
    BF16 = mybir.dt.bfloat16

    with ExitStack() as ctx:
        tc = ctx.enter_context(tile.TileContext(nc))
        singles = ctx.enter_context(tc.tile_pool(name="singles", bufs=1))
        stream = ctx.enter_context(tc.tile_pool(name="stream", bufs=3))
        cache = ctx.enter_context(tc.tile_pool(name="cache", bufs=2))
        mpool = ctx.enter_context(tc.tile_pool(name="mpool", bufs=1))
        psum = ctx.enter_context(tc.tile_pool(name="psum", bufs=1, space="PSUM"))
        psum2 = ctx.enter_context(tc.tile_pool(name="psum2", bufs=2, space="PSUM"))
        small = ctx.enter_context(tc.tile_pool(name="small", bufs=2))

        # Constants (scalar HWDGE ring, so tile loads head the sync ring)
        w_sb = singles.tile([C, CONV_NUM, C], F32)   # [c_in, k, c_out]
        nc.scalar.dma_start(out=w_sb[:], in_=wT[:])
        b_sb = singles.tile([C, CONV_NUM], F32)      # [c, k]
        nc.scalar.dma_start(out=b_sb[:], in_=bT[:])
        a_sb = singles.tile([C, CONV_NUM], F32)      # [c, k] (a1[k] per row)
        nc.scalar.dma_start(out=a_sb[:], in_=aT[:])
        ones_col = singles.tile([P, 1], BF16)
        nc.vector.memset(ones_col[:], 1.0)
        ones_row = singles.tile([1, P], F32)
        nc.vector.memset(ones_row[:], 1.0)
        one_hw = singles.tile([1, 1], F32)
        nc.vector.memset(one_hw[:], 1.0 / HW)
        zeros2 = singles.tile([C, 2], F32)
        nc.vector.memset(zeros2[:], 0.0)
        id_bf = singles.tile([P, P], BF16)
        id_f = singles.tile([C, C], F32)

        # [channel, branch(0=max,1=avg), batch]
        xvec = singles.tile([C, 2, BPC], F32)

        for b in range(BPC):
            # ---- Pass 1(b): stream tiles once; cast; bf16 fold; feed mean ----
            tbs = []
            psum_s = psum2.tile([1, 8 * C], F32, tag="psum_s")
            macc = mpool.tile([P, KF, C], BF16, tag="macc")
            for t in range(T):
                st = stream.tile([P, KF, C], F32, tag="st")
                nc.sync.dma_start(out=st[:], in_=feat_t[b, t])
                if b == 0 and t == 0:
                    # after the first load is queued so it doesn't delay it
                    masks.make_identity(nc, id_bf[:])
                    masks.make_identity(nc, id_f[:])
                # the bf16 cast doubles as the resident pass-2 copy and the
                # max-fold operand
                tb = cache.tile([P, KF, C], BF16, tag=f"tb{t}")
                tbs.append(tb)
                nc.scalar.copy(out=tb[:], in_=st[:])
                if t == 1:
                    nc.vector.tensor_max(macc[:], tbs[0][:], tb[:])
                elif t > 1:
                    nc.vector.tensor_max(macc[:], macc[:], tb[:])
                sv = tb[:].rearrange("p (s r) c -> p s (r c)", s=SEG)
                for seg in range(SEG):
                    nc.tensor.matmul(
                        psum_s[:],
                        ones_col[:],
                        sv[:, seg, :],
                        start=(t == 0 and seg == 0),
                        stop=(t == T - 1 and seg == SEG - 1),
                    )

            # in-place binary tree over the KF axis: (P, KF, C) -> (P, 1, C)
            w = KF
            while w > 1:
                h = w // 2
                nc.vector.tensor_max(
                    macc[:, :h, :], macc[:, :h, :], macc[:, h:w, :]
                )
                w = h
            # cross-partition max: PE transpose (P,C)->(C,P), DVE reduce
            mt = psum.tile([C, P], BF16, tag="mt")
            nc.tensor.transpose(mt[:], macc[:, 0, :], id_bf[:])
            nc.vector.reduce_max(
                out=xvec[:, 0, b : b + 1], in_=mt[:], axis=mybir.AxisListType.X
            )
            # fold (row, channel) mix: (1, C, 8) reduce -> (1, C)
            srow = small.tile([1, C], F32)
            nc.vector.reduce_sum(
                out=srow[:],
                in_=psum_s[:].rearrange("p (s c) -> p c s", c=C),
                axis=mybir.AxisListType.X,
            )
            # transpose row->column via K=1 matmul, folding the 1/HW scale
            av = psum.tile([C, 1], F32, tag="av")
            nc.tensor.matmul(av[:], srow[:], one_hw[:], start=True, stop=True)
            nc.vector.tensor_copy(xvec[:, 1, b : b + 1], av[:])

            # ---- Recurrence(b): 7 residual PReLU blocks on (C, 2) ----
            # prelu(z)+x = (max(z,0)+x) + min(a*z,0) with a>0; 4 DVE ops.
            xf = xvec[:, :, b]  # (C, 2): cols = (max, avg)
            for k in range(CONV_NUM):
                y = psum.tile([C, 2], F32, tag="y")
                nc.tensor.matmul(y[:], w_sb[:, k, :], xf, start=True, stop=True)
                z = small.tile([C, 2], F32)
                nc.vector.tensor_scalar(
                    z[:], y[:], b_sb[:, k : k + 1], 0.0,
                    mybir.AluOpType.add, mybir.AluOpType.bypass,
                )
                px = small.tile([C, 2], F32)
                nc.vector.scalar_tensor_tensor(
                    px[:], z[:], 0.0, xf,
                    mybir.AluOpType.max, mybir.AluOpType.add,
                )
                ng = small.tile([C, 2], F32)
                nc.vector.scalar_tensor_tensor(
                    ng[:], z[:], a_sb[:, k : k + 1], zeros2[:],
                    mybir.AluOpType.mult, mybir.AluOpType.min,
                )
                xn = small.tile([C, 2], F32)
                nc.vector.tensor_add(xn[:], px[:], ng[:])
                xf = xn[:]

            # scores(b) = sigmoid(x_max + x_avg): (C, 1)
            ssum = small.tile([C, 1], F32)
            nc.vector.tensor_add(ssum[:], xf[:, 0:1], xf[:, 1:2])
            scores = small.tile([C, 1], F32)
            nc.scalar.activation(
                out=scores[:], in_=ssum[:], func=mybir.ActivationFunctionType.Sigmoid
            )
            # on-chip broadcast: (C,1) -> (1,C) PE transpose, K=1 ones outer
            # product -> (P, C) PSUM, DVE materializes (P, KF, C) bf16.
            sc_t = psum.tile([1, C], F32, tag="sc_t")
            nc.tensor.transpose(sc_t[:], scores[:], id_f[:])
            sc_sb = small.tile([1, C], F32)
            nc.vector.tensor_copy(sc_sb[:], sc_t[:])
            bc_ps = psum.tile([P, C], F32, tag="bc_ps")
            nc.tensor.matmul(bc_ps[:], ones_row[:], sc_sb[:], start=True, stop=True)
            bc_big = mpool.tile([P, KF, C], BF16, tag="bc_big")
            nc.vector.tensor_copy(
                bc_big[:], bc_ps[:].unsqueeze(1).to_broadcast([P, KF, C])
            )
            # token: gate the NEXT batch's fold chain (WAW on macc) on this
            # batch's finished score row, so full-tile folds never interleave
            # into the serial recurrence above. The corner value is garbage
            # but the next fold overwrites the whole accumulator.
            if b + 1 < BPC:
                nc.vector.tensor_copy(
                    macc[0:1, 0:1, 0:1], sc_sb[0:1, 0:1].unsqueeze(1)
                )

            # ---- Pass 2(b): bf16 scale in place, SWDGE store casts to f32 ----
            for t in range(T):
                nc.vector.tensor_mul(tbs[t][:], tbs[t][:], bc_big[:])
                nc.gpsimd.dma_start(out=out_t[b, t], in_=tbs[t][:])

    _split_dma_waits(nc)
    return nc


def _prep_inputs(features, W1, b1, a1):
    feats = np.ascontiguousarray(features, dtype=np.float32).reshape(B, HW, C)
    # lhsT layout: wT[c_in, k, c_out] = W1[k, c_out, c_in]
    wT = np.ascontiguousarray(np.transpose(np.asarray(W1, np.float32), (2, 0, 1)))
    bT = np.ascontiguousarray(np.asarray(b1, np.float32).T)            # (C, 7)
    aT = np.ascontiguousarray(
        np.broadcast_to(np.asarray(a1, np.float32), (C, CONV_NUM))
    )
    return feats, wT, bT, aT


def kernel(features, W1, b1, a1):
    global LAST_EXEC_NS
    feats, wT, bT, aT = _prep_inputs(features, W1, b1, a1)
    nc = _build_nc()
    in_maps = [
        {
            "features": feats[i * BPC : (i + 1) * BPC],
            "wT": wT,
            "bT": bT,
            "aT": aT,
        }
        for i in range(NCORES)
    ]
    import os

    res = run_bass_kernel_spmd(
        nc,
        in_maps,
        list(range(NCORES)),
        trace=PROFILE,
        tmpdir=os.environ.get("BASS_TMPDIR"),
    )
    global LAST_RESULTS
    LAST_RESULTS = res
    LAST_EXEC_NS = res.exec_time_ns
    out = np.concatenate(
        [res.results[i]["out"].reshape(BPC, H, W, C) for i in range(NCORES)], axis=0
    )
    return out


# revision 20
# speedup vs baseline: 1.1351x; 1.1351x over previous
"""Trainium2 Bass kernel for ChannelFeatures (channel-attention style module).

Computes, per batch element b:
    x_max[b] = max over (H,W) of features[b]          # (C,)
    x_avg[b] = mean over (H,W) of features[b]         # (C,)
    7 residual blocks (shared weights on both branches):
        x = prelu(W1[k] @ x + b1[k], a1[k]) + x
    scores[b] = sigmoid(x_max[b] + x_avg[b])          # (C,)
    out[b] = features[b] * scores[b]                  # broadcast over (H,W)

Sharding: pure data parallel over batch — 16 batch elements across 8 cores,
2 per core, weights replicated. No cross-core communication.

Device strategy per core (2 batch elements, each (65536, 64) fp32):
  HBM traffic is the 2x33.5MB floor (each element loaded once, stored once).
  Loads stream fp32 tiles on the sync HWDGE ring into a small rotating pool;
  ACT casts each tile to bf16 — that cast is BOTH the mean-matmul operand
  (PE ones-matmul accumulation, 1/HW folded into the final K=1 matmul) AND
  the resident pass-2 copy, so both batches fit in SBUF double-buffered and
  a load never waits on a store.
  Max: contiguous DVE bf16 max-folds over the cached casts (2x DVE rate;
  rounding error verified well under the tolerance), in-place binary tree,
  PE transpose + DVE reduce for the cross-partition max. The next batch's
  fold chain is gated on this batch's score row via a 1-element token write
  into the shared accumulator, so the serial recurrence is never dilated by
  interleaved full-tile folds.
  Recurrence: channels on partitions, (C, 2) tile; 7x (PE matmul + fused
  DVE prelu: z = y+b; (z max 0)+x; (a*z min 0); sum).
  Scores broadcast stays on-chip: PE transpose to a row, K=1 ones outer
  product to a (P, C) PSUM block, DVE materializes the (P, KF, C) bf16
  multiplier (no DRAM bounce, no ACT queueing behind the next batch's
  casts).
  Pass 2: bf16 multiply in place (2x DVE rate), store via the gpsimd SWDGE
  ring with a bf16->fp32 cast in the DMA datapath. Loads (sync ring) and
  stores (gpsimd ring) interleave at packet granularity, keeping HBM busy
  through each batch's recurrence gap.
"""

import numpy as np
from contextlib import ExitStack

import concourse.bass as bass
import concourse.tile as tile
from concourse import masks, mybir
from concourse.bass_utils import run_bass_kernel_spmd

# Problem shapes (hardcoded per contract)
B, H, W, C = 16, 256, 256, 64
CONV_NUM = 7
NCORES = 8
BPC = B // NCORES          # batch elements per core
HW = H * W                 # 65536 spatial positions
P = 128                    # SBUF partitions
KF = 32                    # spatial rows per partition per tile
TILE_ROWS = P * KF         # 4096 spatial rows per tile
T = HW // TILE_ROWS        # 16 tiles per batch element
F32 = mybir.dt.float32

# test.py hooks: set PROFILE=True before calling kernel() to capture an NTFF
# trace; LAST_EXEC_NS then holds the max per-core HW execution time.
PROFILE = False
LAST_EXEC_NS = None
LAST_RESULTS = None


def _split_dma_waits(nc: bass.Bass) -> None:
    """The pinned walrus build rejects DMA instructions carrying more than one
    sync-wait ("Too many sync wait commands"). Tile's sem assignment is not
    transitively minimal, so slot-reuse instructions can get two waits
    (consumer release + WAW with the previous writer). Hoist all but the last
    wait onto wait-only EventSemaphore instructions on the same engine right
    before the instruction."""
    n = 0
    # num=200: outside every id Tile allocated (its end-of-kernel range-clear
    # covers the allocated block), so no collision with released Tile sems.
    dummy = nc.alloc_semaphore(name="wsplit_dummy", num=200)
    for fn in nc.m.functions:
        for blk in fn.blocks:
            new_insts = []
            for inst in blk.instructions:
                si = getattr(inst, "sync_info", None)
                if si is not None and len(si.on_wait) > 1:
                    for w in si.on_wait[:-1]:
                        ev = mybir.InstEventSemaphore(
                            name=f"WSPLIT-{n}", ins=[], outs=[]
                        )
                        n += 1
                        ev.engine = inst.engine
                        # Tick a dedicated dummy sem nobody waits on, so the
                        # simulator/race tooling (which require every
                        # instruction to carry an update) accept the carrier.
                        upd = mybir.SyncUpdate(
                            sync_type="semaphore",
                            id=dummy.num,
                            ant_name=dummy.name,
                            update_mode="sem-add-imm",
                            update_value=1,
                        )
                        ev.sync_info = mybir.SyncInfo(on_wait=[w], on_update=[upd])
                        new_insts.append(ev)
                    si.on_wait = [si.on_wait[-1]]
                new_insts.append(inst)
            blk.instructions = new_insts


def _build_nc() -> bass.Bass:
    nc = bass.Bass()
    feat = nc.declare_dram_parameter("features", [BPC, HW, C], F32, isOutput=False)
    wT = nc.declare_dram_parameter("wT", [C, CONV_NUM, C], F32, isOutput=False)
    bT = nc.declare_dram_parameter("bT", [C, CONV_NUM], F32, isOutput=False)
    aT = nc.declare_dram_parameter("aT", [C, CONV_NUM], F32, isOutput=False)
    out = nc.declare_dram_parameter("out", [BPC, HW, C], F32, isOutput=True)

    feat_t = feat[:].rearrange("b (t p k) c -> b t p k c", p=P, k=KF)
    out_t = out[:].rearrange("b (t p k) c -> b t p k c", p=P, k=KF)

    SEG = KF // 8            # 512-wide matmul segments per tile
    BF16 = mybir.dt.bfloat16

    with ExitStack() as ctx:
        tc = ctx.enter_context(tile.TileContext(nc))
        singles = ctx.enter_context(tc.tile_pool(name="singles", bufs=1))
        stream = ctx.enter_context(tc.tile_pool(name="stream", bufs=6))
        cache = ctx.enter_context(tc.tile_pool(name="cache", bufs=2))
        mpool = ctx.enter_context(tc.tile_pool(name="mpool", bufs=1))
        psum = ctx.enter_context(tc.tile_pool(name="psum", bufs=1, space="PSUM"))
        psum2 = ctx.enter_context(tc.tile_pool(name="psum2", bufs=2, space="PSUM"))
        small = ctx.enter_context(tc.tile_pool(name="small", bufs=2))

        # Constants (scalar HWDGE ring, so tile loads head the sync ring)
        w_sb = singles.tile([C, CONV_NUM, C], F32)   # [c_in, k, c_out]
        nc.scalar.dma_start(out=w_sb[:], in_=wT[:])
        b_sb = singles.tile([C, CONV_NUM], F32)      # [c, k]
        nc.scalar.dma_start(out=b_sb[:], in_=bT[:])
        a_sb = singles.tile([C, CONV_NUM], F32)      # [c, k] (a1[k] per row)
        nc.scalar.dma_start(out=a_sb[:], in_=aT[:])
        ones_col = singles.tile([P, 1], BF16)
        nc.vector.memset(ones_col[:], 1.0)
        ones_row = singles.tile([1, P], F32)
        nc.vector.memset(ones_row[:], 1.0)
        one_hw = singles.tile([1, 1], F32)
        nc.vector.memset(one_hw[:], 1.0 / HW)
        zeros2 = singles.tile([C, 2], F32)
        nc.vector.memset(zeros2[:], 0.0)
        id_bf = singles.tile([P, P], BF16)
        id_f = singles.tile([C, C], F32)

        # [channel, branch(0=max,1=avg), batch]
        xvec = singles.tile([C, 2, BPC], F32)

        for b in range(BPC):
            # ---- Pass 1(b): stream tiles once; cast; bf16 fold; feed mean ----
            tbs = []
            psum_s = psum2.tile([1, 8 * C], F32, tag="psum_s")
            macc = mpool.tile([P, KF, C], BF16, tag="macc")
            for t in range(T):
                st = stream.tile([P, KF, C], F32, tag="st")
                nc.sync.dma_start(out=st[:], in_=feat_t[b, t])
                if b == 0 and t == 0:
                    # after the first load is queued so it doesn't delay it
                    masks.make_identity(nc, id_bf[:])
                    masks.make_identity(nc, id_f[:])
                # the bf16 cast doubles as the resident pass-2 copy and the
                # max-fold operand
                tb = cache.tile([P, KF, C], BF16, tag=f"tb{t}")
                tbs.append(tb)
                nc.scalar.copy(out=tb[:], in_=st[:])
                if t == 1:
                    nc.vector.tensor_max(macc[:], tbs[0][:], tb[:])
                elif t > 1:
                    nc.vector.tensor_max(macc[:], macc[:], tb[:])
                sv = tb[:].rearrange("p (s r) c -> p s (r c)", s=SEG)
                for seg in range(SEG):
                    nc.tensor.matmul(
                        psum_s[:],
                        ones_col[:],
                        sv[:, seg, :],
                        start=(t == 0 and seg == 0),
                        stop=(t == T - 1 and seg == SEG - 1),
                    )

            # in-place binary tree over the KF axis: (P, KF, C) -> (P, 1, C)
            w = KF
            while w > 1:
                h = w // 2
                nc.vector.tensor_max(
                    macc[:, :h, :], macc[:, :h, :], macc[:, h:w, :]
                )
                w = h
            # cross-partition max: PE transpose (P,C)->(C,P), DVE reduce
            mt = psum.tile([C, P], BF16, tag="mt")
            nc.tensor.transpose(mt[:], macc[:, 0, :], id_bf[:])
            nc.vector.reduce_max(
                out=xvec[:, 0, b : b + 1], in_=mt[:], axis=mybir.AxisListType.X
            )
            # fold (row, channel) mix: (1, C, 8) reduce -> (1, C)
            srow = small.tile([1, C], F32)
            nc.vector.reduce_sum(
                out=srow[:],
                in_=psum_s[:].rearrange("p (s c) -> p c s", c=C),
                axis=mybir.AxisListType.X,
            )
            # transpose row->column via K=1 matmul, folding the 1/HW scale
            av = psum.tile([C, 1], F32, tag="av")
            nc.tensor.matmul(av[:], srow[:], one_hw[:], start=True, stop=True)
            nc.vector.tensor_copy(xvec[:, 1, b : b + 1], av[:])

            # ---- Recurrence(b): 7 residual PReLU blocks on (C, 2) ----
            # prelu(z)+x = (max(z,0)+x) + min(a*z,0) with a>0; 4 DVE ops.
            xf = xvec[:, :, b]  # (C, 2): cols = (max, avg)
            for k in range(CONV_NUM):
                y = psum.tile([C, 2], F32, tag="y")
                nc.tensor.matmul(y[:], w_sb[:, k, :], xf, start=True, stop=True)
                z = small.tile([C, 2], F32)
                nc.vector.tensor_scalar(
                    z[:], y[:], b_sb[:, k : k + 1], 0.0,
                    mybir.AluOpType.add, mybir.AluOpType.bypass,
                )
                px = small.tile([C, 2], F32)
                nc.vector.scalar_tensor_tensor(
                    px[:], z[:], 0.0, xf,
                    mybir.AluOpType.max, mybir.AluOpType.add,
                )
                ng = small.tile([C, 2], F32)
                nc.vector.scalar_tensor_tensor(
                    ng[:], z[:], a_sb[:, k : k + 1], zeros2[:],
                    mybir.AluOpType.mult, mybir.AluOpType.min,
                )
                xn = small.tile([C, 2], F32)
                nc.vector.tensor_add(xn[:], px[:], ng[:])
                xf = xn[:]

            # scores(b) = sigmoid(x_max + x_avg): (C, 1)
            ssum = small.tile([C, 1], F32)
            nc.vector.tensor_add(ssum[:], xf[:, 0:1], xf[:, 1:2])
            scores = small.tile([C, 1], F32)
            nc.scalar.activation(
                out=scores[:], in_=ssum[:], func=mybir.ActivationFunctionType.Sigmoid
            )
            # on-chip broadcast: (C,1) -> (1,C) PE transpose, K=1 ones outer
            # product -> (P, C) PSUM, DVE materializes (P, KF, C) bf16.
            sc_t = psum.tile([1, C], F32, tag="sc_t")
            nc.tensor.transpose(sc_t[:], scores[:], id_f[:])
            sc_sb = small.tile([1, C], F32)
            nc.vector.tensor_copy(sc_sb[:], sc_t[:])
            bc_ps = psum.tile([P, C], F32, tag="bc_ps")
            nc.tensor.matmul(bc_ps[:], ones_row[:], sc_sb[:], start=True, stop=True)
            bc_big = mpool.tile([P, KF, C], BF16, tag="bc_big")
            nc.vector.tensor_copy(
                bc_big[:], bc_ps[:].unsqueeze(1).to_broadcast([P, KF, C])
            )
            # token: gate the NEXT batch's fold chain (WAW on macc) on this
            # batch's finished score row, so full-tile folds never interleave
            # into the serial recurrence above. The corner value is garbage
            # but the next fold overwrites the whole accumulator.
            if b + 1 < BPC:
                nc.vector.tensor_copy(
                    macc[0:1, 0:1, 0:1], sc_sb[0:1, 0:1].unsqueeze(1)
                )

            # ---- Pass 2(b): bf16 scale in place, SWDGE store casts to f32 ----
            for t in range(T):
                nc.vector.tensor_mul(tbs[t][:], tbs[t][:], bc_big[:])
                nc.gpsimd.dma_start(out=out_t[b, t], in_=tbs[t][:])

    _split_dma_waits(nc)
    return nc


def _prep_inputs(features, W1, b1, a1):
    feats = np.ascontiguousarray(features, dtype=np.float32).reshape(B, HW, C)
    # lhsT layout: wT[c_in, k, c_out] = W1[k, c_out, c_in]
    wT = np.ascontiguousarray(np.transpose(np.asarray(W1, np.float32), (2, 0, 1)))
    bT = np.ascontiguousarray(np.asarray(b1, np.float32).T)            # (C, 7)
    aT = np.ascontiguousarray(
        np.broadcast_to(np.asarray(a1, np.float32), (C, CONV_NUM))
    )
    return feats, wT, bT, aT


def kernel(features, W1, b1, a1):
    global LAST_EXEC_NS
    feats, wT, bT, aT = _prep_inputs(features, W1, b1, a1)
    nc = _build_nc()
    in_maps = [
        {
            "features": feats[i * BPC : (i + 1) * BPC],
            "wT": wT,
            "bT": bT,
            "aT": aT,
        }
        for i in range(NCORES)
    ]
    import os

    res = run_bass_kernel_spmd(
        nc,
        in_maps,
        list(range(NCORES)),
        trace=PROFILE,
        tmpdir=os.environ.get("BASS_TMPDIR"),
    )
    global LAST_RESULTS
    LAST_RESULTS = res
    LAST_EXEC_NS = res.exec_time_ns
    out = np.concatenate(
        [res.results[i]["out"].reshape(BPC, H, W, C) for i in range(NCORES)], axis=0
    )
    return out
